# revision 24
# baseline (speedup 1.0000x reference)
"""Trainium2 Bass kernel for the SAGAN-style self-attention block.

Full-input contract: kernel(**inputs) takes the unsharded numpy inputs and
returns the full-shape output. Internally shards across 8 NeuronCores:
core = (batch_sample, half_of_query_rows).

Math per sample (C=256, Cq=32, N=4096):
    q = (Wq @ F3 + bq) / sqrt(32)        [Cq, N]   (scale folded into q)
    k = Wk @ F1 + bk                     [Cq, N]
    v = gamma * Wv @ F2                  [C, N]    (gamma folded into Wv;
                                                    gamma*bv folded into x3)
    eT[m, n] = sum_c k[c, m] q[c, n]     (energy, transposed layout)
    E = exp(eT)                          (unnormalized attention, transposed)
    U[c, n] = sum_m v[c, m] E[m, n]
    R[n]    = sum_m E[m, n]              (softmax denominator, ones-row matmul)
    y = U / R + (x3 + gamma*bv)

The transposed-energy layout puts the attention contraction dim (m) on
partitions so the big second matmul needs no transposes; softmax
normalization is deferred past the matmul (exp values are bounded: |e| < ~5).
"""

import numpy as np
import ml_dtypes

N_CORES = 8
B, C, HH, WW = 4, 256, 64, 64
N = HH * WW          # 4096 pixels per sample
CQ = 32              # C // 8 query/key channels
NSH = N // 2         # 2048 query rows per core
NT = 512             # free-dim tile (one PSUM bank of fp32)
MC = 128             # contraction chunk (full partition dim)
ISQ = 1.0 / np.sqrt(32.0)

_BF16 = ml_dtypes.bfloat16
_F8 = ml_dtypes.float8_e4m3
_cache = {}


def _build():
    import concourse.tile as tile
    import concourse.mybir as mybir
    from concourse import bacc
    from contextlib import ExitStack

    f32 = mybir.dt.float32
    bf16 = mybir.dt.bfloat16
    f8 = mybir.dt.float8e4
    Act = mybir.ActivationFunctionType
    from concourse.alu_op_type import AluOpType as Alu

    nc = bacc.Bacc("TRN2", target_bir_lowering=False, debug=False,
                   enable_asserts=False, num_devices=N_CORES)

    x3_d = nc.dram_tensor("x3", [C, NSH], f32, kind="ExternalInput").ap()
    # x3b/x1 in DoubleRow pair layout [p, o, n] where channel = 128*o + p
    x3b_d = nc.dram_tensor("x3b", [128, 2, NSH], f8, kind="ExternalInput").ap()
    x1_d = nc.dram_tensor("x1", [C, N], f8, kind="ExternalInput").ap()
    x2_d = nc.dram_tensor("x2", [128, 2, N], f8, kind="ExternalInput").ap()
    wv8_d = nc.dram_tensor("wv8", [128, 2, C], f8, kind="ExternalInput").ap()
    # const blobs: cb [128, 128] fp8 = wqt0|wqt1|wkt0|wkt1; cf = bqs4|bkc4
    cb_d = nc.dram_tensor("cb", [128, 128], f8, kind="ExternalInput").ap()
    cf_d = nc.dram_tensor("cf", [128, 2], f32, kind="ExternalInput").ap()
    y_d = nc.dram_tensor("y", [C, NSH], f32, kind="ExternalOutput").ap()

    n_mc = N // MC            # 32 contraction chunks
    n_nt = NSH // NT          # 4 query-row tiles per core

    with tile.TileContext(nc) as tc, ExitStack() as ctx:
        const = ctx.enter_context(tc.tile_pool(name="const", bufs=1))
        big = ctx.enter_context(tc.tile_pool(name="big", bufs=1))
        ex_pool = ctx.enter_context(tc.tile_pool(name="ex", bufs=8))
        small = ctx.enter_context(tc.tile_pool(name="small", bufs=2))
        ypool = ctx.enter_context(tc.tile_pool(name="y", bufs=2))

        # ---- PE warm-up tile (no DMA dependency) + early exp table load ----
        warm_sb = const.tile([128, NT], bf16, tag="warm", name="warm")
        nc.vector.memset(warm_sb[:], 0.25)
        dact = const.tile([128, 8], f32, tag="dact", name="dact")
        nc.gpsimd.memset(dact[:], 0.0)
        nc.scalar.activation(dact[:], dact[:], Act.Exp)

        # ---- constants / weights ----
        cb_sb = const.tile([128, 128], f8, tag="cb", name="cb")
        cf_sb = const.tile([128, 2], f32, tag="cf", name="cf")
        nc.sync.dma_start(cb_sb[:], cb_d[:])
        nc.sync.dma_start(cf_sb[:], cf_d[:])
        wqt_sb = [cb_sb[:, 32 * i:32 * (i + 1)] for i in range(2)]
        wkt_sb = [cb_sb[:, 64 + 32 * i:64 + 32 * (i + 1)] for i in range(2)]
        bqs_sb = cf_sb[:, 0:1]
        bkc_sb = cf_sb[:, 1:2]
        ones_sb = const.tile([128, 2, 128], f8, tag="ones", name="ones")
        nc.vector.memset(ones_sb[:], 1.0)

        # ---- big activations (chunked DMAs so projections start early) ----
        x3_sb = [big.tile([128, NSH], f32, tag=f"x3_{i}", name=f"x3_{i}") for i in range(2)]
        x3b_sb = big.tile([128, 2, NSH], f8, tag="x3b", name="x3b")
        x1_sb = [big.tile([128, N], f8, tag=f"x1_{i}", name=f"x1_{i}") for i in range(2)]
        x2_sb = big.tile([128, 2, N], f8, tag="x2", name="x2")
        wv8_sb = const.tile([128, 2, C], f8, tag="wv8", name="wv8")
        # interleaved: x1 chunk 0 first (kproj 0-7 needs cols 0:1024), then
        # x3b chunk 0 (qproj 0-1), then the rest
        for i in range(2):
            nc.sync.dma_start(x1_sb[i][:, 0:1024], x1_d[128 * i:128 * (i + 1), 0:1024])
        nc.sync.dma_start(x3b_sb[:, :, 0:1024], x3b_d[:, :, 0:1024])
        for i in range(2):
            nc.sync.dma_start(x1_sb[i][:, 1024:2048], x1_d[128 * i:128 * (i + 1), 1024:2048])
        nc.sync.dma_start(x3b_sb[:, :, 1024:2048], x3b_d[:, :, 1024:2048])
        for c0 in range(2048, N, 1024):
            for i in range(2):
                nc.sync.dma_start(x1_sb[i][:, c0:c0 + 1024],
                                  x1_d[128 * i:128 * (i + 1), c0:c0 + 1024])
        nc.sync.dma_start(wv8_sb[:], wv8_d[:])
        for c0 in range(0, N, 1024):
            nc.sync.dma_start(x2_sb[:, :, c0:c0 + 1024], x2_d[:, :, c0:c0 + 1024])
        # (x3 fp32 residual DMA deferred until after k-projection)

        # q4: q replicated in all 4 partition quadrants [32r+ck, n]
        # k4: chunk jj of k at partition quadrant jj%4, col block jj//4
        q4_sb = big.tile([128, NSH], bf16, tag="q4", name="q4")
        k4_sb = big.tile([128, N // 4], bf16, tag="k4", name="k4")
        vt_sb = big.tile([128, n_mc, C], f8, tag="vt", name="vt")  # [m in chunk, chunk, c]

        psum_e = ctx.enter_context(tc.tile_pool(name="psum_e", bufs=1, space="PSUM"))
        proj_ctx = ExitStack()
        psum_p = proj_ctx.enter_context(
            tc.tile_pool(name="psum_p", bufs=3, space="PSUM"))

        # PE warm-up while input DMAs stream: HAM un-throttles after ~3.4us of
        # sustained matmul activity; runs on a memset tile (no DMA dep) so the
        # real projections start at 2.4 GHz instead of 1.2.
        warm = psum_p.tile([128, NT], f32, tag="pj", name="pj")
        for w in range(10):
            nc.tensor.matmul(warm[:, :], warm_sb[:, :128],
                             warm_sb[:, :], start=True, stop=True)
        nc.vector.tensor_copy(warm[:1, :1], warm[:1, :1])  # keep a reader

        DR = mybir.MatmulPerfMode.DoubleRow

        # ---- projections ----
        def kproj(j):
            # k chunks 4j..4j+3 -> quadrant layout via col-group tiling
            kp4 = psum_p.tile([128, MC], f32, tag="pj", name="pj")
            for r in range(4):
                jj = 4 * j + r
                nc.tensor.matmul(kp4[32 * r:32 * (r + 1), :], wkt_sb[0][:],
                                 x1_sb[0][:, MC * jj:MC * (jj + 1)],
                                 start=True, stop=False, tile_position=(0, 32 * r))
                nc.tensor.matmul(kp4[32 * r:32 * (r + 1), :], wkt_sb[1][:],
                                 x1_sb[1][:, MC * jj:MC * (jj + 1)],
                                 start=False, stop=True, tile_position=(0, 32 * r))
            nc.vector.tensor_scalar_add(k4_sb[:, MC * j:MC * (j + 1)], kp4[:],
                                        bkc_sb[:])

        def qproj(j):
            # q n-tile j replicated into all 4 partition quadrants
            qp = psum_p.tile([128, NT], f32, tag="pj", name="pj")
            for r in range(4):
                nc.tensor.matmul(qp[32 * r:32 * (r + 1), :], wqt_sb[0][:],
                                 x3b_sb[:, 0, NT * j:NT * (j + 1)],
                                 start=True, stop=False, tile_position=(0, 32 * r))
                nc.tensor.matmul(qp[32 * r:32 * (r + 1), :], wqt_sb[1][:],
                                 x3b_sb[:, 1, NT * j:NT * (j + 1)],
                                 start=False, stop=True, tile_position=(0, 32 * r))
            nc.vector.tensor_scalar(q4_sb[:, NT * j:NT * (j + 1)], qp[:],
                                    ISQ, bqs_sb[:], Alu.mult, Alu.add)

        def vtproj(i):
            # vT[m, c] for m-chunk pair (i, i+1): plain fp8 with channel-half
            # accumulation. The [128,128] x2 stationaries are FWL-eligible
            # (DoubleRow would disable fast-weight-load and expose a 256-col
            # LDWEIGHTS per matmul).
            vp = psum_p.tile([128, 2, C], f32, tag="pj", name="pj")
            for u in range(2):
                for o in range(2):
                    nc.tensor.matmul(vp[:, u, :],
                                     x2_sb[:, o, MC * (i + u):MC * (i + u + 1)],
                                     wv8_sb[:, o, :],
                                     start=(o == 0), stop=(o == 1))
            nc.vector.tensor_copy(vt_sb[:, i:i + 2, :], vp[:])

        kproj(0)
        qproj(0)

        # ---- attention main loop ----
        n_grp = n_mc // 4  # 8 groups per n-tile
        groups = [(t, g) for t in range(n_nt) for g in range(n_grp)]
        utiles = {}
        pendq = []

        # Schraudolph constants: fp8-e4m3 bits of exp(e) ~= e*8*log2(e) + 56.34
        # (computed on DVE as f32->int8 with rounding, bitcast to fp8). The
        # softmax ratio cancels the ~3% per-element error the same way it
        # cancels fp8 quantization error. Offloads part of the exp stream
        # from ScalarE (the steady-state bottleneck) to the Vector engine.
        SCH_A = 11.541560327111707   # 8 / ln(2)
        SCH_B = 56.344               # 7*8 + 0.344 centering

        def emit_group(t, g):
            eps = [psum_e.tile([128, 2, NT], f32, tag=h, name=h) for h in ("pea", "peb")]
            for r in range(4):
                nc.tensor.matmul(eps[r // 2][:, r % 2, :],
                                 k4_sb[32 * r:32 * (r + 1), MC * g:MC * (g + 1)],
                                 q4_sb[32 * r:32 * (r + 1), NT * t:NT * (t + 1)],
                                 start=True, stop=True,
                                 tile_position=(32 * r, 0))
            sch = (t >= 1) and (g in (1, 3, 5))
            exa = ex_pool.tile([128, 2, NT], f8, tag="exa", name="exa")
            nc.scalar.activation(exa[:], eps[0][:], Act.Exp)
            if sch:
                exb = ex_pool.tile([128, 2, NT], mybir.dt.int8, tag="exs", name="exs")
                nc.vector.tensor_scalar(exb[:], eps[1][:], SCH_A, SCH_B,
                                        Alu.mult, Alu.add)
                pendq.append(([exa[:], exb[:].bitcast(f8)], t, g))
            else:
                exb = ex_pool.tile([128, 2, NT], f8, tag="exb", name="exb")
                nc.scalar.activation(exb[:], eps[1][:], Act.Exp)
                pendq.append(([exa[:], exb[:]], t, g))

        def u_block(ex4, t, g):
            if g == 0:
                utiles[t] = (
                    psum_a.tile([128, NT], f32, tag="u0", name="u0"),
                    psum_a.tile([128, NT], f32, tag="u1", name="u1"),
                    psum_a.tile([128, NT], f32, tag="rr", name="rr", bufs=2),
                )
            u0, u1, rr = utiles[t]
            for pr in range(2):
                jj = 4 * g + 2 * pr            # pair covers chunks jj, jj+1
                st, sp = (jj == 0), (jj == n_mc - 2)
                exp_pair = ex4[pr]
                nc.tensor.matmul(u0[:], vt_sb[:, jj:jj + 2, 0:128], exp_pair,
                                 start=st, stop=sp, perf_mode=DR)
                nc.tensor.matmul(u1[:], vt_sb[:, jj:jj + 2, 128:C], exp_pair,
                                 start=st, stop=sp, perf_mode=DR)
                nc.tensor.matmul(rr[:], ones_sb[:], exp_pair, start=st, stop=sp,
                                 perf_mode=DR)
            if g == n_grp - 1:
                epilogue(t)

        def epilogue(t):
            # y = U/R + x3' (gamma already folded into Wv, gamma*bv into x3').
            # On the last n-tile the chain is exposed in the kernel tail, so
            # process it in half-tiles (DMA overlaps DVE) with the ct0 adds
            # on GpSimd.
            last = (t == n_nt - 1)
            u0, u1, rr = utiles.pop(t)
            us = (u0, u1)
            rec = small.tile([128, NT], f32, tag="rec", name="rec")
            if not last:
                nc.vector.reciprocal_approx_fast(rec[:], rr[:])
                ys = [ypool.tile([128, NT], f32, tag=f"ys{ct}", name=f"ys{ct}")
                      for ct in range(2)]
                nc.vector.tensor_mul(ys[0][:], u0[:], rec[:])
                nc.vector.tensor_mul(ys[1][:], u1[:], rec[:])
                for ct in range(2):
                    ys2 = ypool.tile([128, NT], f32, tag=f"ys2{ct}", name=f"ys2{ct}")
                    nc.vector.tensor_add(ys2[:], ys[ct][:],
                                         x3_sb[ct][:, NT * t:NT * (t + 1)])
                    nc.sync.dma_start(
                        y_d[128 * ct:128 * (ct + 1), NT * t:NT * (t + 1)], ys2[:])
                return
            NH = NT // 2
            for p in range(2):
                sl = slice(NH * p, NH * (p + 1))
                nc.vector.reciprocal_approx_fast(rec[:, sl], rr[:, sl])
                for ct in range(2):
                    ys = ypool.tile([128, NH], f32, tag=f"yl{ct}", name=f"yl{ct}")
                    nc.vector.tensor_mul(ys[:], us[ct][:, sl], rec[:, sl])
                    ys2 = ypool.tile([128, NH], f32, tag=f"yl2{ct}", name=f"yl2{ct}")
                    eng = nc.gpsimd if ct == 0 else nc.vector
                    eng.tensor_add(ys2[:], ys[:],
                                   x3_sb[ct][:, NT * t + NH * p:NT * t + NH * (p + 1)])
                    nc.sync.dma_start(
                        y_d[128 * ct:128 * (ct + 1),
                            NT * t + NH * p:NT * t + NH * (p + 1)], ys2[:])

        # 7 exp groups banked before/during the remaining projections keep
        # ScalarE streaming (accumulators not needed yet); kproj(g) is
        # emitted just-in-time before the group that uses it.
        for (t, g) in groups[:7]:
            emit_group(t, g)
            if g + 1 <= 7:
                kproj(g + 1)
        for i in range(0, n_mc, 2):
            vtproj(i)
        for i in range(2):
            nc.vector.tensor_copy(x3_sb[i][:1, :1], k4_sb[:1, MC * 7:MC * 7 + 1])
            nc.sync.dma_start(x3_sb[i][:], x3_d[128 * i:128 * (i + 1), :])
        for j in range(1, 4):
            qproj(j)
        proj_ctx.close()
        psum_a = ctx.enter_context(tc.tile_pool(name="psum_a", bufs=1, space="PSUM"))

        for (t, g) in groups[7:]:
            emit_group(t, g)
            while len(pendq) > 5:
                u_block(*pendq.pop(0))
        while pendq:
            u_block(*pendq.pop(0))

    nc.compile()
    return nc


def _get_nc():
    if "nc" not in _cache:
        _cache["nc"] = _build()
    return _cache["nc"]


def kernel(F3, F1, F2, Wq, bq, Wk, bk, Wv, bv, gamma):
    from concourse import bass_utils

    nc = _get_nc()

    gam = np.float32(np.asarray(gamma).reshape(()))
    F3 = np.asarray(F3, dtype=np.float32)
    r3 = F3.reshape(B, C, N)
    r1 = np.asarray(F1, dtype=np.float32).reshape(B, C, N)
    r2 = np.asarray(F2, dtype=np.float32).reshape(B, C, N)
    wqt = np.asarray(Wq, np.float32).T
    wkt = np.asarray(Wk, np.float32).T
    wvt = np.asarray(Wv, np.float32).T * gam
    cb = np.empty((128, 128), np.float32)
    cb[:, 0:32] = wqt[:128]; cb[:, 32:64] = wqt[128:]
    cb[:, 64:96] = wkt[:128]; cb[:, 96:128] = wkt[128:]
    cb = cb.astype(_F8)
    cf = np.empty((128, 2), np.float32)
    cf[:, 0] = np.tile(np.asarray(bq, np.float32) * ISQ, 4)
    cf[:, 1] = np.tile(np.asarray(bk, np.float32), 4)
    wv8 = np.ascontiguousarray(
        wvt.reshape(2, 128, C).transpose(1, 0, 2)).astype(_F8)
    # residual with gamma*bv folded in
    x3r = r3 + (gam * np.asarray(bv, np.float32))[None, :, None]
    in_maps = []
    for cid in range(N_CORES):
        b, h = divmod(cid, 2)
        x3h = np.ascontiguousarray(r3[b][:, NSH * h:NSH * (h + 1)])
        in_maps.append({
            "x3": np.ascontiguousarray(x3r[b][:, NSH * h:NSH * (h + 1)]),
            "x3b": np.ascontiguousarray(
                x3h.reshape(2, 128, NSH).transpose(1, 0, 2)).astype(_F8),
            "x1": r1[b].astype(_F8),
            "x2": np.ascontiguousarray(
                r2[b].reshape(2, 128, N).transpose(1, 0, 2)).astype(_F8),
            "wv8": wv8,
            "cb": cb, "cf": cf,
        })

    _cache["in_maps"] = in_maps
    res = bass_utils.run_bass_kernel_spmd(nc, in_maps, core_ids=list(range(N_CORES)))
    out = np.empty((B, C, N), np.float32)
    for cid in range(N_CORES):
        b, h = divmod(cid, 2)
        out[b][:, NSH * h:NSH * (h + 1)] = res.results[cid]["y"]
    return out.reshape(B, C, HH, WW)
